# revision 22
# baseline (speedup 1.0000x reference)
import sys

import numpy as np

sys.path.insert(0, "/opt/trn_rl_repo")

TRACE = False
LAST = {}
_cache = {}

SPARSITY = 0.5


def _tf32(a):
    b = np.ascontiguousarray(np.asarray(a, np.float32))
    u = b.view(np.uint32).copy()
    u += np.uint32(0x0FFF) + ((u >> np.uint32(13)) & np.uint32(1))
    u &= np.uint32(0xFFFFE000)
    return u.view(np.float32)


def _masked(w, s):
    sa = np.abs(np.asarray(s, np.float32)).ravel()
    j = int((1.0 - SPARSITY) * sa.size)
    thr = np.partition(sa, j)[j]
    m = (np.abs(np.asarray(s, np.float32)) >= thr).astype(np.float32)
    return (np.asarray(w, np.float32) * m).astype(np.float32)


def _prep(inputs):
    w1m = _masked(inputs["w1"], inputs["s1"])  # [128,3,3,3]
    w2m = _masked(inputs["w2"], inputs["s2"])  # [256,128,3,3]
    w3m = _masked(inputs["w3"], inputs["s3"])  # [512,256,3,3]
    fw1m = _masked(inputs["fw1"], inputs["fs1"])  # [1024,512]
    fw2m = _masked(inputs["fw2"], inputs["fs2"])  # [10,1024]

    c = np.ascontiguousarray
    # conv1 as single K=27 matmul: lhsT [27(ch,ky,kx), 128]
    w1t = c(w1m.transpose(1, 2, 3, 0).reshape(27, 128))
    w2t = c(w2m.transpose(1, 2, 3, 0).reshape(128, 9 * 256))
    w3t = c(w3m.reshape(512, 2, 128, 3, 3).transpose(2, 1, 3, 4, 0).reshape(128, 2, 9 * 512))
    # global-avg-pool 1/256 folded into fw1
    fw1t = c((fw1m.T.reshape(4, 128, 1024).transpose(1, 0, 2) / 256.0).astype(np.float32))
    fw2t = c(fw2m.T.reshape(8, 128, 10).transpose(1, 0, 2))

    weights = {
        "w1t": _tf32(w1t),
        "w2t": _tf32(w2t),
        "w3t": _tf32(w3t),
        "fw1t": fw1t,
        "fw2t": fw2t,
        "b1": c(np.asarray(inputs["b1"], np.float32).reshape(128, 1)),
        "b2": c(np.asarray(inputs["b2"], np.float32).reshape(2, 128).T),
        "b3": c(np.asarray(inputs["b3"], np.float32).reshape(4, 128).T),
        "fb1": c(np.asarray(inputs["fb1"], np.float32).reshape(8, 128).T),
        "fb2": c(np.asarray(inputs["fb2"], np.float32).reshape(10, 1)),
    }
    xpad = np.zeros((64, 3, 66, 66), np.float32)
    xpad[:, :, 1:65, 1:65] = _tf32(inputs["x"])
    # im2col over (ch,ky,kx): x27[i, ch*9+ky*3+kx] = xpad[i, ch, ky:ky+64, kx:kx+64]
    x27 = np.empty((64, 27, 64, 64), np.float32)
    for ch in range(3):
        for ky in range(3):
            for kx in range(3):
                x27[:, ch * 9 + ky * 3 + kx] = xpad[:, ch, ky : ky + 64, kx : kx + 64]
    return x27, weights


def _build():
    import concourse.bacc as bacc
    import concourse.mybir as mybir
    import concourse.tile as tile

    FP = mybir.dt.float32
    FR = mybir.dt.float32r
    RELU = mybir.ActivationFunctionType.Relu

    nc = bacc.Bacc("TRN2", target_bir_lowering=False, debug=False)

    xpad_d = nc.dram_tensor("xpad", [8, 27, 64, 64], FR, kind="ExternalInput")
    w1t_d = nc.dram_tensor("w1t", [27, 128], FR, kind="ExternalInput")
    w2t_d = nc.dram_tensor("w2t", [128, 2304], FR, kind="ExternalInput")
    w3t_d = nc.dram_tensor("w3t", [128, 2, 4608], FR, kind="ExternalInput")
    fw1t_d = nc.dram_tensor("fw1t", [128, 4, 1024], FP, kind="ExternalInput")
    fw2t_d = nc.dram_tensor("fw2t", [128, 8, 10], FP, kind="ExternalInput")
    b1_d = nc.dram_tensor("b1", [128, 1], FP, kind="ExternalInput")
    b2_d = nc.dram_tensor("b2", [128, 2], FP, kind="ExternalInput")
    b3_d = nc.dram_tensor("b3", [128, 4], FP, kind="ExternalInput")
    fb1_d = nc.dram_tensor("fb1", [128, 8], FP, kind="ExternalInput")
    fb2_d = nc.dram_tensor("fb2", [10, 1], FP, kind="ExternalInput")
    outT_d = nc.dram_tensor("outT", [10, 8], FP, kind="ExternalOutput")

    with tile.TileContext(nc) as tc:
        with tc.tile_pool(name="consts", bufs=1) as consts, \
             tc.tile_pool(name="xim_p", bufs=2) as xim_p, \
             tc.tile_pool(name="act_p", bufs=1) as act_p, \
             tc.tile_pool(name="h3_p", bufs=2) as h3_p, \
             tc.tile_pool(name="ps1_p", bufs=2, space="PSUM") as ps1_p, \
             tc.tile_pool(name="ps2_p", bufs=2, space="PSUM") as ps2_p, \
             tc.tile_pool(name="ps3_p", bufs=2, space="PSUM") as ps3_p, \
             tc.tile_pool(name="psf_p", bufs=1, space="PSUM") as psf_p:

            w1t = consts.tile([27, 128], FR)
            nc.sync.dma_start(out=w1t[:, :], in_=w1t_d[:, :])
            b1sb = consts.tile([128, 1], FP)
            nc.sync.dma_start(out=b1sb[:, :], in_=b1_d[:, :])
            w2t = consts.tile([128, 2304], FR)
            nc.sync.dma_start(out=w2t[:, :], in_=w2t_d[:, :])
            b2sb = consts.tile([128, 2], FP)
            nc.sync.dma_start(out=b2sb[:, :], in_=b2_d[:, :])
            b3sb = consts.tile([128, 4], FP)
            nc.sync.dma_start(out=b3sb[:, :], in_=b3_d[:, :])
            fb1sb = consts.tile([128, 8], FP)
            nc.sync.dma_start(out=fb1sb[:, :], in_=fb1_d[:, :])
            fb2sb = consts.tile([10, 1], FP)
            nc.sync.dma_start(out=fb2sb[:, :], in_=fb2_d[:, :])
            fw2t = consts.tile([128, 8, 10], FP)
            nc.sync.dma_start(out=fw2t[:, :, :], in_=fw2t_d[:, :, :])
            w3t = consts.tile([128, 2, 4608], FR)
            for kt in range(2):
                nc.sync.dma_start(out=w3t[:, kt, 0:2304], in_=w3t_d[:, kt, 0:2304])
                nc.sync.dma_start(out=w3t[:, kt, 2304:4608], in_=w3t_d[:, kt, 2304:4608])
            fw1t = consts.tile([128, 4, 1024], FP)
            for kt in range(4):
                nc.sync.dma_start(out=fw1t[:, kt, :], in_=fw1t_d[:, kt, :])

            h1pad_a = act_p.tile([128, 66, 66], FR)
            h1pad_b = act_p.tile([128, 66, 66], FR)
            h2pad_a = act_p.tile([128, 2, 2, 34, 34], FR)
            h2pad_b = act_p.tile([128, 2, 2, 34, 34], FR)
            hpool = act_p.tile([128, 4, 8], FP)
            z1T = act_p.tile([128, 8, 8], FP)
            y_sb = act_p.tile([10, 8], FP)

            nc.vector.memset(h1pad_a[:, :, :].bitcast(FP), 0.0)
            nc.vector.memset(h1pad_b[:, :, :].bitcast(FP), 0.0)
            for m in range(2):
                nc.vector.memset(h2pad_a[:, m, :, :, :].bitcast(FP), 0.0)
                nc.vector.memset(h2pad_b[:, m, :, :, :].bitcast(FP), 0.0)

            h1pads = [h1pad_a, h1pad_b]
            h2pads = [h2pad_a, h2pad_b]

            def conv1(img, h1pad):
                xim = xim_p.tile([27, 64, 64], FR, name="xim")
                nc.sync.dma_start(out=xim[:, :, :], in_=xpad_d[img, :, :, :])
                for nt in range(8):
                    ps = ps1_p.tile([128, 8, 64], FP, name="ps_c1")
                    nc.tensor.matmul(
                        out=ps[:, :, :],
                        lhsT=w1t[:, :],
                        rhs=xim[:, 8 * nt : 8 * nt + 8, :],
                        start=True,
                        stop=True,
                    )
                    nc.scalar.activation(
                        out=h1pad[:, 1 + 8 * nt : 9 + 8 * nt, 1:65],
                        in_=ps[:, :, :],
                        func=RELU,
                        bias=b1sb[:, 0:1],
                    )

            def conv2(img, h1pad, h2pad, islot):
                for m in range(2):
                    for nh in range(2):
                        ps = ps2_p.tile([128, 16, 32], FP, name="ps_c2")
                        for g in range(9):
                            ky, kx = g // 3, g % 3
                            nc.tensor.matmul(
                                out=ps[:, :, :],
                                lhsT=w2t[:, 256 * g + 128 * m : 256 * g + 128 * m + 128],
                                rhs=h1pad[:, 32 * nh + ky : 32 * nh + ky + 32 : 2, kx : kx + 64 : 2],
                                start=(g == 0),
                                stop=(g == 8),
                            )
                        nc.scalar.activation(
                            out=h2pad[:, m, islot, 1 + 16 * nh : 17 + 16 * nh, 1:33],
                            in_=ps[:, :, :],
                            func=RELU,
                            bias=b2sb[:, m : m + 1],
                        )

            def conv3(pair, h2pad):
                for mt in range(4):
                    ps = ps3_p.tile([128, 2, 16, 16], FP, name="ps_c3")
                    n = 0
                    for kt in range(2):
                        for g in range(9):
                            ky, kx = g // 3, g % 3
                            nc.tensor.matmul(
                                out=ps[:, :, :, :],
                                lhsT=w3t[:, kt, 512 * g + 128 * mt : 512 * g + 128 * mt + 128],
                                rhs=h2pad[:, kt, :, ky : ky + 32 : 2, kx : kx + 32 : 2],
                                start=(n == 0),
                                stop=(n == 17),
                            )
                            n += 1
                    h3 = h3_p.tile([128, 2, 16, 16], FP, name="h3scr")
                    for i in range(2):
                        nc.scalar.activation(
                            out=h3[:, i, :, :],
                            in_=ps[:, i, :, :],
                            func=RELU,
                            bias=b3sb[:, mt : mt + 1],
                            accum_out=hpool[:, mt, 2 * pair + i : 2 * pair + i + 1],
                        )

            for pair in range(4):
                h2pad = h2pads[pair % 2]
                for i in range(2):
                    img = 2 * pair + i
                    h1pad = h1pads[img % 2]
                    conv1(img, h1pad)
                    conv2(img, h1pad, h2pad, i)
                conv3(pair, h2pad)

            for m in range(8):
                psz = psf_p.tile([128, 8], FP, name="psz")
                for kt in range(4):
                    nc.tensor.matmul(
                        out=psz[:, :],
                        lhsT=fw1t[:, kt, 128 * m : 128 * m + 128],
                        rhs=hpool[:, kt, :],
                        start=(kt == 0),
                        stop=(kt == 3),
                    )
                nc.scalar.activation(
                    out=z1T[:, m, :],
                    in_=psz[:, :],
                    func=RELU,
                    bias=fb1sb[:, m : m + 1],
                )

            psy = psf_p.tile([10, 8], FP, name="psy")
            for kt in range(8):
                nc.tensor.matmul(
                    out=psy[:, :],
                    lhsT=fw2t[:, kt, :],
                    rhs=z1T[:, kt, :],
                    start=(kt == 0),
                    stop=(kt == 7),
                )
            nc.vector.tensor_scalar_add(y_sb[:, :], psy[:, :], fb2sb[:, 0:1])
            nc.sync.dma_start(out=outT_d[:, :], in_=y_sb[:, :])

    nc.compile()
    return nc


def _get_nc():
    if "nc" not in _cache:
        _cache["nc"] = _build()
    return _cache["nc"]


def kernel(**inputs):
    from concourse import bass_utils

    nc = _get_nc()
    xpad, weights = _prep(inputs)
    in_maps = [
        dict(weights, xpad=np.ascontiguousarray(xpad[8 * c : 8 * c + 8]))
        for c in range(8)
    ]
    res = bass_utils.run_bass_kernel_spmd(
        nc, in_maps, core_ids=list(range(8)), trace=TRACE
    )
    LAST["exec_time_ns"] = getattr(res, "exec_time_ns", None)
    LAST["profile_json"] = getattr(res, "profile_json", None)
    LAST["instructions_and_trace"] = getattr(res, "instructions_and_trace", None)
    out = np.concatenate([r["outT"].T for r in res.results], axis=0)
    return np.ascontiguousarray(out.astype(np.float32))
